# revision 73
# baseline (speedup 1.0000x reference)
"""AFT-simple attention (nn_AsfAttention) on 8 TRN2 NeuronCores.

Reference (per batch b):
    emb_q = q @ w_q; emb_k = k @ w_k; emb_v = v @ w_v
    k_exp = exp(emb_k)
    y = sigmoid(emb_q) * cumsum(k_exp * emb_v, seq) / cumsum(k_exp, seq)
    out = y @ w_p

Sharding: core c = 2*b + h handles batch b, seq half h (4096 rows each).
The cumsum carry across the half boundary is a [H]-vector pair exchanged
via an AllReduce over two 4-core replica groups (each group contains
both halves of two batches) of a slot-masked [128, 64] buffer — far
lower latency than the all-8 variant; fully pairwise groups crash the
runtime.  The collective's staging DMAs ride the GpSimd software DGE so
they are FIFO-ordered with the collective and never queue behind spills.

Layout: everything channel-major on chip ([ch partitions, seq free]).
The q/k projections run as fp8-e4m3 DoubleRow matmuls (2 fp8 pairs per
PE cell, K=256 per instruction): inputs ship as fp8 [128, K, seq] with
k-subtiles in the free dim, weights as fp8 [128, K, H] scaled by 256
(undone via the activation scale).  v/p projections stay bf16 — fp8
there fails the 2e-2 accuracy budget (measured rel err: bf16-all 0.005,
q+k fp8 0.0135, +v or +p fp8 >0.037).  The seq cumsums are DVE
tensor_tensor_scans chained across chunks; prefix tensors spill to DRAM
in [part, m, seq] layout between the two phases and reload as one DMA
per chunk.  Next-chunk input loads are emitted before each chunk's
compute-gated spill writes so the in-order DMA queue never stalls the
projections, and the phase-B q/spill loads are prefetched mid-phase-A.
"""

import sys

import numpy as np
import ml_dtypes

import concourse.bass as bass
import concourse.tile as tile
from concourse import bacc, mybir
from concourse.bass_utils import run_bass_kernel_spmd

B, S, H = 4, 8192, 1024
NCORES = 8
SC_CORE = S // 2          # 4096 seq rows per core
SC = 512                  # seq chunk (columns per matmul / scan step)
NS = SC_CORE // SC        # 8 seq chunks
M = H // 128              # 8 ch_out tiles
K = H // 128              # 8 ch_in tiles
KP = K // 2               # 4 DoubleRow k-pairs for the fp8 projections
NT = SC // 128            # 4 seq subtiles per chunk for the out projection
W8SCALE = 256.0           # fp8 weight pre-scale (undone in the activation)

bf16 = mybir.dt.bfloat16
f32 = mybir.dt.float32
f8 = mybir.dt.float8e4
AF = mybir.ActivationFunctionType
OP = mybir.AluOpType
DR = mybir.MatmulPerfMode.DoubleRow

_cache = {}


def prune_pe_incs(nc, verbose=False):
    """Drop the per-matmul PE semaphore increment from every matmul that
    is not the end of its accumulation group (stop_tensor_calc), then
    renumber all waits on that semaphore. A wait whose original target
    inc was dropped is rounded UP to the next kept inc — safe here
    because kept-inc matmuls never depend on the rounded-up waiters
    (they only consume earlier-generation tiles).

    Each inc is a serialized EVT_SEM register write (~26 ns) on the PE
    sequencer; at 2048 matmuls this is ~45 us of PE issue overhead.
    """
    import bisect
    from collections import defaultdict

    insts = []
    for bb in nc.main_func.blocks:
        insts.extend(bb.instructions)

    upd = defaultdict(list)
    for pos, ins in enumerate(insts):
        si = ins.sync_info
        if not si:
            continue
        for u in si.on_update:
            upd[u.ant_name].append((pos, ins, u))

    changed = 0
    for sem, us in upd.items():
        if len(us) < 64:
            continue
        if not all(type(i).__name__ == "InstMatmult"
                   and u.update_mode == "sem-inc" and u.update_value == 1
                   for _, i, u in us):
            continue
        kept = [bool(i.stop_tensor_calc) for _, i, _ in us]
        kept[-1] = True
        kept_idx = [j for j, k in enumerate(kept) if k]

        def new_thresh(t):
            j = bisect.bisect_left(kept_idx, t - 1)
            assert j < len(kept_idx), f"wait {t} beyond last kept inc"
            return j + 1

        for ins in insts:
            si = ins.sync_info
            if not si:
                continue
            for w in si.on_wait:
                if w.ant_name == sem:
                    assert w.wait_mode == "sem-ge-imm", w.wait_mode
                    w.wait_value = new_thresh(w.wait_value)
        for j, (_, ins, u) in enumerate(us):
            if not kept[j]:
                si = ins.sync_info
                rest = [x for x in si.on_update if x.ant_name != sem]
                assert len(rest) == len(si.on_update) - 1
                if rest:
                    si.on_update[:] = rest
                else:
                    si.on_update.clear()
                changed += 1
    if verbose:
        print(f"prune_pe_incs: removed {changed} matmul sem-incs")
    return changed


def build(ns=NS, debug=False):
    sc_core = ns * SC
    nc = bacc.Bacc("TRN2", target_bir_lowering=False, debug=debug,
                   num_devices=NCORES)

    # q/k go through fp8 DoubleRow projections: [part, ksub, seq] layout
    # so a [:, 2p:2p+2, :] slice is one 256-deep DoubleRow contraction.
    qT_e = nc.dram_tensor("qT", [128, K, sc_core], f8, kind="ExternalInput")
    kT_e = nc.dram_tensor("kT", [128, K, sc_core], f8, kind="ExternalInput")
    vT_e = nc.dram_tensor("vT", [128, K, sc_core], bf16,
                          kind="ExternalInput")
    wq_e = nc.dram_tensor("wq", [128, K, H], f8, kind="ExternalInput")
    wk_e = nc.dram_tensor("wk", [128, K, H], f8, kind="ExternalInput")
    wv_e = nc.dram_tensor("wv", [H, H], bf16, kind="ExternalInput")
    wp_e = nc.dram_tensor("wp", [H, H], bf16, kind="ExternalInput")
    # slot masks for the carry exchange: the AllReduce runs in two
    # 4-core groups (each containing both halves of two batches), so
    # totals live in a [128, 4*16] buffer (4 group slots x (2 quantities
    # x 8 m-tiles)).
    NGRP = 4
    sself_e = nc.dram_tensor("slot_self", [128, NGRP * 2 * M], f32,
                             kind="ExternalInput")
    spart_e = nc.dram_tensor("slot_partner", [128, NGRP * 2 * M], f32,
                             kind="ExternalInput")
    out_e = nc.dram_tensor("out", [sc_core, H], f32, kind="ExternalOutput")

    # spills in [part, m, seq] layout so each phase-B reload is ONE DMA
    sk_sp = nc.dram_tensor("sk_sp", [128, K, sc_core], bf16)
    skv_sp = nc.dram_tensor("skv_sp", [128, K, sc_core], bf16)
    tot_in = nc.dram_tensor("tot_in", [128, NGRP * 2 * M], f32)
    tot_out = nc.dram_tensor("tot_out", [128, NGRP * 2 * M], f32)

    with tile.TileContext(nc) as tc:
        with (
            tc.tile_pool(name="wts", bufs=2) as wts,
            tc.tile_pool(name="inb", bufs=2) as inb,
            tc.tile_pool(name="act", bufs=2) as actp,
            tc.tile_pool(name="kvp", bufs=2) as kvp,
            tc.tile_pool(name="ldp", bufs=2) as ldp,
            tc.tile_pool(name="scn", bufs=2) as scn,
            tc.tile_pool(name="tmp", bufs=3) as tmp,
            tc.tile_pool(name="osb", bufs=3) as osbp,
            tc.tile_pool(name="sml", bufs=1) as sml,
            tc.tile_pool(name="ps", bufs=4, space="PSUM") as ps,
            tc.tile_pool(name="pso", bufs=2, space="PSUM") as pso,
        ):
            def load_w(ext, tagpfx):
                # column-halved loads: the first half unblocks the m=0-3
                # projections after 1 MiB instead of 2
                ts_ = []
                for kk in range(K):
                    t = wts.tile([128, H], bf16, tag=f"{tagpfx}{kk}")
                    nc.sync.dma_start(t[:, 0:512],
                                      ext[kk * 128:(kk + 1) * 128, 0:512])
                    ts_.append(t)
                for kk in range(K):
                    nc.sync.dma_start(ts_[kk][:, 512:1024],
                                      ext[kk * 128:(kk + 1) * 128, 512:1024])
                return ts_

            def load_in8(ext, tagpfx, c):
                t = inb.tile([128, K, SC], f8, tag=tagpfx)
                nc.sync.dma_start(t[:], ext[:, :, bass.ts(c, SC)])
                return t

            def load_inv(c):
                t = inb.tile([128, K, SC], bf16, tag="iv8")
                nc.sync.dma_start(t[:], vT_e[:, :, bass.ts(c, SC)])
                return t

            def load_sp(s):
                # one DMA each for the chunk's sk/skv reloads
                lsk = ldp.tile([128, K, SC], bf16, tag="lsk")
                nc.sync.dma_start(lsk[:], sk_sp[:, :, bass.ts(s, SC)])
                lskv = ldp.tile([128, K, SC], bf16, tag="lskv")
                nc.sync.dma_start(lskv[:], skv_sp[:, :, bass.ts(s, SC)])
                return lsk, lskv

            def proj(w_t, iv8, m):
                psm = ps.tile([128, SC], f32, tag="ps")
                for kk in range(K):
                    nc.tensor.matmul(
                        psm[:], w_t[kk][:, m * 128:(m + 1) * 128],
                        iv8[:, kk, :], start=(kk == 0), stop=(kk == K - 1))
                return psm

            def proj8(w_t, in_t, m):
                psm = ps.tile([128, SC], f32, tag="ps")
                for kp in range(KP):
                    ks = slice(2 * kp, 2 * kp + 2)
                    nc.tensor.matmul(
                        psm[:], w_t[:, ks, m * 128:(m + 1) * 128],
                        in_t[:, ks, :], start=(kp == 0), stop=(kp == KP - 1),
                        perf_mode=DR)
                return psm

            # Load order matters for the head: the first matmul group
            # needs wk cols 0-511 + the first k chunk, so those go first.
            wk_t = wts.tile([128, K, H], f8, tag="wa8", name="wk8")
            nc.sync.dma_start(wk_t[:, :, 0:512], wk_e[:, :, 0:512])
            kc0 = load_in8(kT_e, "ik8", 0)
            nc.sync.dma_start(wk_t[:, :, 512:1024], wk_e[:, :, 512:1024])
            vc0 = load_inv(0)
            wv_t = load_w(wv_e, "wb")
            sself = sml.tile([128, NGRP * 2 * M], f32, tag="sself")
            spart = sml.tile([128, NGRP * 2 * M], f32, tag="spart")
            nc.sync.dma_start(sself[:], sself_e[:])
            nc.sync.dma_start(spart[:], spart_e[:])

            # ---- phase A: k/v projections, exp, kv-mult, scans, spills
            wq_t = None
            wp_t = []
            sk_prev = [None] * M
            skv_prev = [None] * M
            qpre = {}
            lds = {}
            kc, vc = kc0, vc0
            for s in range(ns):
                ssl = bass.ts(s, SC)
                # next chunk's input loads go into the DMA queue BEFORE
                # this chunk's compute-gated spill writes, so the queue
                # never stalls the next chunk's projections.
                if s + 1 < ns:
                    kc_n = load_in8(kT_e, "ik8", s + 1)
                    vc_n = load_inv(s + 1)
                ke = []
                for m in range(M):
                    psm = proj8(wk_t, kc, m)
                    t = actp.tile([128, SC], bf16, tag=f"ke{m}")
                    nc.scalar.activation(t[:], psm[:], AF.Exp,
                                         scale=1.0 / W8SCALE)
                    ke.append(t)
                for m in range(M):
                    psm = proj(wv_t, vc, m)
                    # Scalar (idle in phase A) bridges emb_v out of PSUM
                    # so the kv multiply runs on GpSimd and the DVE only
                    # carries the scans — otherwise the DVE paces the
                    # whole of phase A via PSUM-bank recycling.
                    ev = tmp.tile([128, SC], bf16, tag="ev")
                    nc.scalar.copy(ev[:], psm[:])
                    kv = kvp.tile([128, SC], bf16, tag=f"kv{m}")
                    nc.gpsimd.tensor_mul(kv[:], ke[m][:], ev[:])
                    # skv scans first: their tails feed the collective
                    skvt = scn.tile([128, SC], bf16, tag=f"sv{m}")
                    init = 0.0 if s == 0 else skv_prev[m][:, SC - 1:SC]
                    nc.vector.tensor_tensor_scan(
                        skvt[:], kv[:], kv[:], init, OP.add, OP.bypass)
                    skv_prev[m] = skvt
                    nc.sync.dma_start(skv_sp[:, m, ssl], skvt[:])
                    skt = scn.tile([128, SC], bf16, tag=f"sk{m}")
                    init = 0.0 if s == 0 else sk_prev[m][:, SC - 1:SC]
                    nc.vector.tensor_tensor_scan(
                        skt[:], ke[m][:], ke[m][:], init, OP.add, OP.bypass)
                    sk_prev[m] = skt
                    nc.sync.dma_start(sk_sp[:, m, ssl], skt[:])
                # phase-B weights spread across early chunks; phase-B
                # input prefetches (q chunks 0-1, spill reloads 0-1)
                # emitted mid-phase-A so their DMAs run long before the
                # boundary instead of queueing behind chunk-7 spills.
                if s == 0:
                    wq_t = wts.tile([128, K, H], f8, tag="wa8", name="wq8")
                    nc.sync.dma_start(wq_t[:, :, 0:512], wq_e[:, :, 0:512])
                elif s == 1:
                    nc.sync.dma_start(wq_t[:, :, 512:1024],
                                      wq_e[:, :, 512:1024])
                elif s == 3:
                    qpre[0] = load_in8(qT_e, "iq8", 0)
                elif s == 4:
                    qpre[1] = load_in8(qT_e, "iq8", 1)
                    lds[0] = load_sp(0)
                elif s == 5:
                    lds[1] = load_sp(1)
                for kk in range(s * K // ns, (s + 1) * K // ns):
                    t = wts.tile([128, H], bf16, tag=f"wb{kk}",
                                 name=f"wp{kk}")
                    nc.sync.dma_start(t[:], wp_e[kk * 128:(kk + 1) * 128, :])
                    wp_t.append(t)
                if s + 1 < ns:
                    kc, vc = kc_n, vc_n

            # ---- carry exchange: totals -> slots -> allreduce -> carry
            NSLOT = 2 * M
            tot = sml.tile([128, NSLOT], f32, tag="tot")
            for m in range(M):
                nc.vector.tensor_copy(tot[:, 2 * m:2 * m + 1],
                                      sk_prev[m][:, SC - 1:SC])
                nc.vector.tensor_copy(tot[:, 2 * m + 1:2 * m + 2],
                                      skv_prev[m][:, SC - 1:SC])
            slots = sml.tile([128, NGRP * NSLOT], f32, tag="slots")
            for c in range(NGRP):
                nc.vector.tensor_copy(
                    slots[:, c * NSLOT:(c + 1) * NSLOT], tot[:])
            nc.vector.tensor_mul(slots[:], slots[:], sself[:])
            # software-DGE DMAs: FIFO-ordered with the collective on the
            # GpSimd queue, bypassing the (spill-congested) HWDGE queue
            nc.gpsimd.dma_start(tot_in[:], slots[:])
            nc.gpsimd.collective_compute(
                "AllReduce", OP.add,
                replica_groups=[[0, 1, 2, 3], [4, 5, 6, 7]],
                ins=[tot_in.ap().opt()],
                outs=[tot_out.ap().opt()],
            )
            alltot = sml.tile([128, NGRP * NSLOT], f32, tag="alltot")
            nc.gpsimd.dma_start(alltot[:], tot_out[:])
            nc.vector.tensor_mul(alltot[:], alltot[:], spart[:])
            r2 = sml.tile([128, 2 * NSLOT], f32, tag="r2")
            nc.vector.tensor_add(r2[:], alltot[:, 0:2 * NSLOT],
                                 alltot[:, 2 * NSLOT:4 * NSLOT])
            carry = sml.tile([128, NSLOT], f32, tag="carry")
            nc.vector.tensor_add(carry[:], r2[:, 0:NSLOT],
                                 r2[:, NSLOT:2 * NSLOT])

            # ---- phase B: EQ + sigmoid, final elementwise, out
            #      projection.  Emission skewed so EQ(s+skew) PE work is
            #      queued ahead of OUT(s) to ride through the collective.
            def emit_eq(s):
                qc = qpre.pop(s) if s in qpre else load_in8(qT_e, "iq8", s)
                sg = []
                for m in range(M):
                    psm = proj8(wq_t, qc, m)
                    t = actp.tile([128, SC], bf16, tag=f"ke{m}")
                    nc.scalar.activation(t[:], psm[:], AF.Sigmoid,
                                         scale=1.0 / W8SCALE)
                    sg.append(t)
                return sg

            def emit_out(s, sg, ld):
                lsk, lskv = ld
                ys = []
                for m in range(M):
                    den = tmp.tile([128, SC], f32, tag="den")
                    nc.scalar.activation(den[:], lsk[:, m, :], AF.Identity,
                                         bias=carry[:, 2 * m:2 * m + 1])
                    num = tmp.tile([128, SC], f32, tag="num")
                    nc.scalar.activation(num[:], lskv[:, m, :], AF.Identity,
                                         bias=carry[:, 2 * m + 1:2 * m + 2])
                    rcp = tmp.tile([128, SC], f32, tag="num")
                    nc.vector.reciprocal_approx_fast(rcp[:], den[:])
                    rat = tmp.tile([128, SC], f32, tag="den")
                    nc.vector.tensor_mul(rat[:], num[:], rcp[:])
                    y = scn.tile([128, SC], bf16, tag=f"sv{m}")
                    nc.vector.tensor_mul(y[:], rat[:], sg[m][:])
                    ys.append(y)
                for t4 in range(NT):
                    psm = pso.tile([128, 1024], f32)
                    tsl = bass.ts(t4, 128)
                    for m in range(M):
                        for n in range(2):
                            nc.tensor.matmul(
                                psm[:, n * 512:(n + 1) * 512],
                                ys[m][:, tsl],
                                wp_t[m][:, n * 512:(n + 1) * 512],
                                start=(m == 0), stop=(m == M - 1))
                    ob = osbp.tile([128, 1024], f32, tag="ob")
                    nc.scalar.copy(ob[:], psm[:])
                    # alternate output writes across both HWDGE queues so
                    # the final flush drains at double bandwidth
                    eng = nc.sync if t4 % 2 == 0 else nc.scalar
                    eng.dma_start(
                        out_e[s * SC + t4 * 128:s * SC + (t4 + 1) * 128, :],
                        ob[:])

            skew = min(2, ns)
            states = {}
            for s in range(skew):
                states[s] = emit_eq(s)

            def ensure_ld(i):
                if i < ns and i not in lds:
                    lds[i] = load_sp(i)

            for s in range(skew, ns):
                o = s - skew
                ensure_ld(o + 2)
                emit_out(o, states.pop(o), lds.pop(o))
                states[s] = emit_eq(s)
            for s in range(ns - skew, ns):
                ensure_ld(s + 2)
                emit_out(s, states.pop(s), lds.pop(s))

    nc.compile()
    prune_pe_incs(nc, verbose=True)
    return nc


def _to8(x):
    # [H, n] -> [128, K, n] fp8 (k-subtile p-major layout for DoubleRow)
    f8np = ml_dtypes.float8_e4m3
    return np.ascontiguousarray(
        np.asarray(x, dtype=f8np).reshape(K, 128, -1).transpose(1, 0, 2))


def _to3(x):
    # [H, n] -> [128, K, n] bf16 (k-subtile p-major layout)
    return np.ascontiguousarray(
        np.asarray(x, dtype=ml_dtypes.bfloat16)
        .reshape(K, 128, -1).transpose(1, 0, 2))


def _in_maps(q, k, v, w_q, w_k, w_v, w_p):
    bf = ml_dtypes.bfloat16
    ws = {
        "wq": _to8(np.clip(w_q * W8SCALE, -240, 240)),
        "wk": _to8(np.clip(w_k * W8SCALE, -240, 240)),
        "wv": np.ascontiguousarray(w_v, dtype=bf),
        "wp": np.ascontiguousarray(w_p, dtype=bf),
    }
    NSLOT = 2 * M
    NGRP = 4
    in_maps = []
    for c in range(NCORES):
        b, h = c // 2, c % 2
        sl = slice(h * SC_CORE, (h + 1) * SC_CORE)
        g = c % 4  # slot within the 4-core replica group
        sself = np.zeros((128, NGRP * NSLOT), np.float32)
        sself[:, g * NSLOT:(g + 1) * NSLOT] = 1.0
        spart = np.zeros((128, NGRP * NSLOT), np.float32)
        if h == 1:
            p = g ^ 1
            spart[:, p * NSLOT:(p + 1) * NSLOT] = 1.0
        in_maps.append({
            "qT": _to8(q[b, sl].T),
            "kT": _to8(k[b, sl].T),
            "vT": _to3(v[b, sl].T),
            **ws,
            "slot_self": sself,
            "slot_partner": spart,
        })
    return in_maps


def run(q, k, v, w_q, w_k, w_v, w_p, trace=False, tmpdir=None):
    if "nc" not in _cache:
        _cache["nc"] = build()
    nc = _cache["nc"]
    in_maps = _in_maps(q, k, v, w_q, w_k, w_v, w_p)
    res = run_bass_kernel_spmd(nc, in_maps, core_ids=list(range(NCORES)),
                               trace=trace, tmpdir=tmpdir)
    out = np.empty((B, S, H), np.float32)
    for c in range(NCORES):
        b, h = c // 2, c % 2
        out[b, h * SC_CORE:(h + 1) * SC_CORE, :] = res.results[c]["out"]
    return out, res


def kernel(**inputs):
    out, _ = run(**{k: np.asarray(v) for k, v in inputs.items()})
    return out


# revision 74
# speedup vs baseline: 1.0109x; 1.0109x over previous
"""AFT-simple attention (nn_AsfAttention) on 8 TRN2 NeuronCores.

Reference (per batch b):
    emb_q = q @ w_q; emb_k = k @ w_k; emb_v = v @ w_v
    k_exp = exp(emb_k)
    y = sigmoid(emb_q) * cumsum(k_exp * emb_v, seq) / cumsum(k_exp, seq)
    out = y @ w_p

Sharding: core c = 2*b + h handles batch b, seq half h (4096 rows each).
The cumsum carry across the half boundary is a [H]-vector pair exchanged
via an AllReduce over two 4-core replica groups (each group contains
both halves of two batches) of a slot-masked [128, 64] buffer — far
lower latency than the all-8 variant; fully pairwise groups crash the
runtime.  The collective's staging DMAs ride the GpSimd software DGE so
they are FIFO-ordered with the collective and never queue behind spills.

Layout: everything channel-major on chip ([ch partitions, seq free]).
The q/k projections run as fp8-e4m3 DoubleRow matmuls (2 fp8 pairs per
PE cell, K=256 per instruction): inputs ship as fp8 [128, K, seq] with
k-subtiles in the free dim, weights as fp8 [128, K, H] scaled by 256
(undone via the activation scale).  v/p projections stay bf16 — fp8
there fails the 2e-2 accuracy budget (measured rel err: bf16-all 0.005,
q+k fp8 0.0135, +v or +p fp8 >0.037).  The seq cumsums are DVE
tensor_tensor_scans chained across chunks; prefix tensors spill to DRAM
in [part, m, seq] layout between the two phases and reload as one DMA
per chunk.  Next-chunk input loads are emitted before each chunk's
compute-gated spill writes so the in-order DMA queue never stalls the
projections, and the phase-B q/spill loads are prefetched mid-phase-A.
"""

import sys

import numpy as np
import ml_dtypes

import concourse.bass as bass
import concourse.tile as tile
from concourse import bacc, mybir
from concourse.bass_utils import run_bass_kernel_spmd

B, S, H = 4, 8192, 1024
NCORES = 8
SC_CORE = S // 2          # 4096 seq rows per core
SC = 512                  # seq chunk (columns per matmul / scan step)
NS = SC_CORE // SC        # 8 seq chunks
M = H // 128              # 8 ch_out tiles
K = H // 128              # 8 ch_in tiles
KP = K // 2               # 4 DoubleRow k-pairs for the fp8 projections
NT = SC // 128            # 4 seq subtiles per chunk for the out projection
W8SCALE = 256.0           # fp8 weight pre-scale (undone in the activation)

bf16 = mybir.dt.bfloat16
f32 = mybir.dt.float32
f8 = mybir.dt.float8e4
AF = mybir.ActivationFunctionType
OP = mybir.AluOpType
DR = mybir.MatmulPerfMode.DoubleRow

_cache = {}


def prune_pe_incs(nc, verbose=False):
    """Drop the per-matmul PE semaphore increment from every matmul that
    is not the end of its accumulation group (stop_tensor_calc), then
    renumber all waits on that semaphore. A wait whose original target
    inc was dropped is rounded UP to the next kept inc — safe here
    because kept-inc matmuls never depend on the rounded-up waiters
    (they only consume earlier-generation tiles).

    Each inc is a serialized EVT_SEM register write (~26 ns) on the PE
    sequencer; at 2048 matmuls this is ~45 us of PE issue overhead.
    """
    import bisect
    from collections import defaultdict

    insts = []
    for bb in nc.main_func.blocks:
        insts.extend(bb.instructions)

    upd = defaultdict(list)
    for pos, ins in enumerate(insts):
        si = ins.sync_info
        if not si:
            continue
        for u in si.on_update:
            upd[u.ant_name].append((pos, ins, u))

    changed = 0
    for sem, us in upd.items():
        if len(us) < 64:
            continue
        if not all(type(i).__name__ == "InstMatmult"
                   and u.update_mode == "sem-inc" and u.update_value == 1
                   for _, i, u in us):
            continue
        kept = [bool(i.stop_tensor_calc) for _, i, _ in us]
        kept[-1] = True
        kept_idx = [j for j, k in enumerate(kept) if k]

        def new_thresh(t):
            j = bisect.bisect_left(kept_idx, t - 1)
            assert j < len(kept_idx), f"wait {t} beyond last kept inc"
            return j + 1

        for ins in insts:
            si = ins.sync_info
            if not si:
                continue
            for w in si.on_wait:
                if w.ant_name == sem:
                    assert w.wait_mode == "sem-ge-imm", w.wait_mode
                    w.wait_value = new_thresh(w.wait_value)
        for j, (_, ins, u) in enumerate(us):
            if not kept[j]:
                si = ins.sync_info
                rest = [x for x in si.on_update if x.ant_name != sem]
                assert len(rest) == len(si.on_update) - 1
                if rest:
                    si.on_update[:] = rest
                else:
                    si.on_update.clear()
                changed += 1
    if verbose:
        print(f"prune_pe_incs: removed {changed} matmul sem-incs")
    return changed


def build(ns=NS, debug=False):
    sc_core = ns * SC
    nc = bacc.Bacc("TRN2", target_bir_lowering=False, debug=debug,
                   num_devices=NCORES)

    # q/k go through fp8 DoubleRow projections: [part, ksub, seq] layout
    # so a [:, 2p:2p+2, :] slice is one 256-deep DoubleRow contraction.
    qT_e = nc.dram_tensor("qT", [128, K, sc_core], f8, kind="ExternalInput")
    kT_e = nc.dram_tensor("kT", [128, K, sc_core], f8, kind="ExternalInput")
    vT_e = nc.dram_tensor("vT", [128, K, sc_core], bf16,
                          kind="ExternalInput")
    wq_e = nc.dram_tensor("wq", [128, K, H], f8, kind="ExternalInput")
    wk_e = nc.dram_tensor("wk", [128, K, H], f8, kind="ExternalInput")
    wv_e = nc.dram_tensor("wv", [H, H], bf16, kind="ExternalInput")
    wp_e = nc.dram_tensor("wp", [H, H], bf16, kind="ExternalInput")
    # slot masks for the carry exchange: the AllReduce runs in two
    # 4-core groups (each containing both halves of two batches), so
    # totals live in a [128, 4*16] buffer (4 group slots x (2 quantities
    # x 8 m-tiles)).
    NGRP = 4
    sself_e = nc.dram_tensor("slot_self", [128, NGRP * 2 * M], f32,
                             kind="ExternalInput")
    spart_e = nc.dram_tensor("slot_partner", [128, NGRP * 2 * M], f32,
                             kind="ExternalInput")
    out_e = nc.dram_tensor("out", [sc_core, H], f32, kind="ExternalOutput")

    # spills in [part, m, seq] layout so each phase-B reload is ONE DMA
    sk_sp = nc.dram_tensor("sk_sp", [128, K, sc_core], bf16)
    skv_sp = nc.dram_tensor("skv_sp", [128, K, sc_core], bf16)
    tot_in = nc.dram_tensor("tot_in", [128, NGRP * 2 * M], f32)
    tot_out = nc.dram_tensor("tot_out", [128, NGRP * 2 * M], f32)

    with tile.TileContext(nc) as tc:
        with (
            tc.tile_pool(name="wts", bufs=2) as wts,
            tc.tile_pool(name="inb", bufs=2) as inb,
            tc.tile_pool(name="act", bufs=2) as actp,
            tc.tile_pool(name="kvp", bufs=2) as kvp,
            tc.tile_pool(name="ldp", bufs=2) as ldp,
            tc.tile_pool(name="scn", bufs=2) as scn,
            tc.tile_pool(name="tmp", bufs=3) as tmp,
            tc.tile_pool(name="osb", bufs=3) as osbp,
            tc.tile_pool(name="sml", bufs=1) as sml,
            tc.tile_pool(name="ps", bufs=4, space="PSUM") as ps,
            tc.tile_pool(name="pso", bufs=2, space="PSUM") as pso,
        ):
            def load_w(ext, tagpfx):
                # column-halved loads: the first half unblocks the m=0-3
                # projections after 1 MiB instead of 2
                ts_ = []
                for kk in range(K):
                    t = wts.tile([128, H], bf16, tag=f"{tagpfx}{kk}")
                    nc.sync.dma_start(t[:, 0:512],
                                      ext[kk * 128:(kk + 1) * 128, 0:512])
                    ts_.append(t)
                for kk in range(K):
                    nc.sync.dma_start(ts_[kk][:, 512:1024],
                                      ext[kk * 128:(kk + 1) * 128, 512:1024])
                return ts_

            def load_in8(ext, tagpfx, c):
                t = inb.tile([128, K, SC], f8, tag=tagpfx)
                nc.sync.dma_start(t[:], ext[:, :, bass.ts(c, SC)])
                return t

            def load_inv(c):
                t = inb.tile([128, K, SC], bf16, tag="iv8")
                nc.sync.dma_start(t[:], vT_e[:, :, bass.ts(c, SC)])
                return t

            def load_sp(s):
                # one DMA each for the chunk's sk/skv reloads
                lsk = ldp.tile([128, K, SC], bf16, tag="lsk")
                nc.sync.dma_start(lsk[:], sk_sp[:, :, bass.ts(s, SC)])
                lskv = ldp.tile([128, K, SC], bf16, tag="lskv")
                nc.sync.dma_start(lskv[:], skv_sp[:, :, bass.ts(s, SC)])
                return lsk, lskv

            def proj(w_t, iv8, m):
                psm = ps.tile([128, SC], f32, tag="ps")
                for kk in range(K):
                    nc.tensor.matmul(
                        psm[:], w_t[kk][:, m * 128:(m + 1) * 128],
                        iv8[:, kk, :], start=(kk == 0), stop=(kk == K - 1))
                return psm

            def proj8(w_t, in_t, m):
                psm = ps.tile([128, SC], f32, tag="ps")
                for kp in range(KP):
                    ks = slice(2 * kp, 2 * kp + 2)
                    nc.tensor.matmul(
                        psm[:], w_t[:, ks, m * 128:(m + 1) * 128],
                        in_t[:, ks, :], start=(kp == 0), stop=(kp == KP - 1),
                        perf_mode=DR)
                return psm

            # Load order matters for the head: the first matmul group
            # needs wk cols 0-511 + the first k chunk, so those go first.
            wk_t = wts.tile([128, K, H], f8, tag="wa8", name="wk8")
            nc.sync.dma_start(wk_t[:, :, 0:512], wk_e[:, :, 0:512])
            kc0 = load_in8(kT_e, "ik8", 0)
            nc.sync.dma_start(wk_t[:, :, 512:1024], wk_e[:, :, 512:1024])
            vc0 = load_inv(0)
            wv_t = load_w(wv_e, "wb")
            sself = sml.tile([128, NGRP * 2 * M], f32, tag="sself")
            spart = sml.tile([128, NGRP * 2 * M], f32, tag="spart")
            nc.sync.dma_start(sself[:], sself_e[:])
            nc.sync.dma_start(spart[:], spart_e[:])

            # ---- phase A: k/v projections, exp, kv-mult, scans, spills
            wq_t = None
            wp_t = []
            sk_prev = [None] * M
            skv_prev = [None] * M
            qpre = {}
            lds = {}
            kc, vc = kc0, vc0
            for s in range(ns):
                ssl = bass.ts(s, SC)
                # next chunk's input loads go into the DMA queue BEFORE
                # this chunk's compute-gated spill writes, so the queue
                # never stalls the next chunk's projections.
                if s + 1 < ns:
                    kc_n = load_in8(kT_e, "ik8", s + 1)
                    vc_n = load_inv(s + 1)
                ke = []
                for m in range(M):
                    psm = proj8(wk_t, kc, m)
                    t = actp.tile([128, SC], bf16, tag=f"ke{m}")
                    nc.scalar.activation(t[:], psm[:], AF.Exp,
                                         scale=1.0 / W8SCALE)
                    ke.append(t)
                for m in range(M):
                    psm = proj(wv_t, vc, m)
                    kv = kvp.tile([128, SC], bf16, tag=f"kv{m}")
                    nc.vector.tensor_mul(kv[:], ke[m][:], psm[:])
                    # skv scans first: their tails feed the collective
                    skvt = scn.tile([128, SC], bf16, tag=f"sv{m}")
                    init = 0.0 if s == 0 else skv_prev[m][:, SC - 1:SC]
                    nc.vector.tensor_tensor_scan(
                        skvt[:], kv[:], kv[:], init, OP.add, OP.bypass)
                    skv_prev[m] = skvt
                    nc.sync.dma_start(skv_sp[:, m, ssl], skvt[:])
                    skt = scn.tile([128, SC], bf16, tag=f"sk{m}")
                    init = 0.0 if s == 0 else sk_prev[m][:, SC - 1:SC]
                    nc.vector.tensor_tensor_scan(
                        skt[:], ke[m][:], ke[m][:], init, OP.add, OP.bypass)
                    sk_prev[m] = skt
                    nc.sync.dma_start(sk_sp[:, m, ssl], skt[:])
                # phase-B weights spread across early chunks; phase-B
                # input prefetches (q chunks 0-1, spill reloads 0-1)
                # emitted mid-phase-A so their DMAs run long before the
                # boundary instead of queueing behind chunk-7 spills.
                if s == 0:
                    wq_t = wts.tile([128, K, H], f8, tag="wa8", name="wq8")
                    nc.sync.dma_start(wq_t[:, :, 0:512], wq_e[:, :, 0:512])
                elif s == 1:
                    nc.sync.dma_start(wq_t[:, :, 512:1024],
                                      wq_e[:, :, 512:1024])
                elif s == 3:
                    qpre[0] = load_in8(qT_e, "iq8", 0)
                elif s == 4:
                    qpre[1] = load_in8(qT_e, "iq8", 1)
                    lds[0] = load_sp(0)
                elif s == 5:
                    lds[1] = load_sp(1)
                for kk in range(s * K // ns, (s + 1) * K // ns):
                    t = wts.tile([128, H], bf16, tag=f"wb{kk}",
                                 name=f"wp{kk}")
                    nc.sync.dma_start(t[:], wp_e[kk * 128:(kk + 1) * 128, :])
                    wp_t.append(t)
                if s + 1 < ns:
                    kc, vc = kc_n, vc_n

            # ---- carry exchange: totals -> slots -> allreduce -> carry
            NSLOT = 2 * M
            tot = sml.tile([128, NSLOT], f32, tag="tot")
            for m in range(M):
                nc.vector.tensor_copy(tot[:, 2 * m:2 * m + 1],
                                      sk_prev[m][:, SC - 1:SC])
                nc.vector.tensor_copy(tot[:, 2 * m + 1:2 * m + 2],
                                      skv_prev[m][:, SC - 1:SC])
            slots = sml.tile([128, NGRP * NSLOT], f32, tag="slots")
            for c in range(NGRP):
                nc.vector.tensor_copy(
                    slots[:, c * NSLOT:(c + 1) * NSLOT], tot[:])
            nc.vector.tensor_mul(slots[:], slots[:], sself[:])
            # software-DGE DMAs: FIFO-ordered with the collective on the
            # GpSimd queue, bypassing the (spill-congested) HWDGE queue
            nc.gpsimd.dma_start(tot_in[:], slots[:])
            nc.gpsimd.collective_compute(
                "AllReduce", OP.add,
                replica_groups=[[0, 1, 2, 3], [4, 5, 6, 7]],
                ins=[tot_in.ap().opt()],
                outs=[tot_out.ap().opt()],
            )
            alltot = sml.tile([128, NGRP * NSLOT], f32, tag="alltot")
            nc.gpsimd.dma_start(alltot[:], tot_out[:])
            nc.vector.tensor_mul(alltot[:], alltot[:], spart[:])
            r2 = sml.tile([128, 2 * NSLOT], f32, tag="r2")
            nc.vector.tensor_add(r2[:], alltot[:, 0:2 * NSLOT],
                                 alltot[:, 2 * NSLOT:4 * NSLOT])
            carry = sml.tile([128, NSLOT], f32, tag="carry")
            nc.vector.tensor_add(carry[:], r2[:, 0:NSLOT],
                                 r2[:, NSLOT:2 * NSLOT])

            # ---- phase B: EQ + sigmoid, final elementwise, out
            #      projection.  Emission skewed so EQ(s+skew) PE work is
            #      queued ahead of OUT(s) to ride through the collective.
            def emit_eq(s):
                qc = qpre.pop(s) if s in qpre else load_in8(qT_e, "iq8", s)
                sg = []
                for m in range(M):
                    psm = proj8(wq_t, qc, m)
                    t = actp.tile([128, SC], bf16, tag=f"ke{m}")
                    nc.scalar.activation(t[:], psm[:], AF.Sigmoid,
                                         scale=1.0 / W8SCALE)
                    sg.append(t)
                return sg

            def emit_out(s, sg, ld):
                lsk, lskv = ld
                ys = []
                for m in range(M):
                    den = tmp.tile([128, SC], f32, tag="den")
                    nc.scalar.activation(den[:], lsk[:, m, :], AF.Identity,
                                         bias=carry[:, 2 * m:2 * m + 1])
                    num = tmp.tile([128, SC], f32, tag="num")
                    nc.scalar.activation(num[:], lskv[:, m, :], AF.Identity,
                                         bias=carry[:, 2 * m + 1:2 * m + 2])
                    rcp = tmp.tile([128, SC], f32, tag="num")
                    nc.vector.reciprocal_approx_fast(rcp[:], den[:])
                    rat = tmp.tile([128, SC], f32, tag="den")
                    nc.vector.tensor_mul(rat[:], num[:], rcp[:])
                    y = scn.tile([128, SC], bf16, tag=f"sv{m}")
                    nc.vector.tensor_mul(y[:], rat[:], sg[m][:])
                    ys.append(y)
                for t4 in range(NT):
                    psm = pso.tile([128, 1024], f32)
                    tsl = bass.ts(t4, 128)
                    for m in range(M):
                        for n in range(2):
                            nc.tensor.matmul(
                                psm[:, n * 512:(n + 1) * 512],
                                ys[m][:, tsl],
                                wp_t[m][:, n * 512:(n + 1) * 512],
                                start=(m == 0), stop=(m == M - 1))
                    ob = osbp.tile([128, 1024], f32, tag="ob")
                    nc.scalar.copy(ob[:], psm[:])
                    nc.sync.dma_start(
                        out_e[s * SC + t4 * 128:s * SC + (t4 + 1) * 128, :],
                        ob[:])

            skew = min(2, ns)
            states = {}
            for s in range(skew):
                states[s] = emit_eq(s)

            def ensure_ld(i):
                if i < ns and i not in lds:
                    lds[i] = load_sp(i)

            for s in range(skew, ns):
                o = s - skew
                ensure_ld(o + 2)
                emit_out(o, states.pop(o), lds.pop(o))
                states[s] = emit_eq(s)
            for s in range(ns - skew, ns):
                ensure_ld(s + 2)
                emit_out(s, states.pop(s), lds.pop(s))

    nc.compile()
    prune_pe_incs(nc, verbose=True)
    return nc


def _to8(x):
    # [H, n] -> [128, K, n] fp8 (k-subtile p-major layout for DoubleRow)
    f8np = ml_dtypes.float8_e4m3
    return np.ascontiguousarray(
        np.asarray(x, dtype=f8np).reshape(K, 128, -1).transpose(1, 0, 2))


def _to3(x):
    # [H, n] -> [128, K, n] bf16 (k-subtile p-major layout)
    return np.ascontiguousarray(
        np.asarray(x, dtype=ml_dtypes.bfloat16)
        .reshape(K, 128, -1).transpose(1, 0, 2))


def _in_maps(q, k, v, w_q, w_k, w_v, w_p):
    bf = ml_dtypes.bfloat16
    ws = {
        "wq": _to8(np.clip(w_q * W8SCALE, -240, 240)),
        "wk": _to8(np.clip(w_k * W8SCALE, -240, 240)),
        "wv": np.ascontiguousarray(w_v, dtype=bf),
        "wp": np.ascontiguousarray(w_p, dtype=bf),
    }
    NSLOT = 2 * M
    NGRP = 4
    in_maps = []
    for c in range(NCORES):
        b, h = c // 2, c % 2
        sl = slice(h * SC_CORE, (h + 1) * SC_CORE)
        g = c % 4  # slot within the 4-core replica group
        sself = np.zeros((128, NGRP * NSLOT), np.float32)
        sself[:, g * NSLOT:(g + 1) * NSLOT] = 1.0
        spart = np.zeros((128, NGRP * NSLOT), np.float32)
        if h == 1:
            p = g ^ 1
            spart[:, p * NSLOT:(p + 1) * NSLOT] = 1.0
        in_maps.append({
            "qT": _to8(q[b, sl].T),
            "kT": _to8(k[b, sl].T),
            "vT": _to3(v[b, sl].T),
            **ws,
            "slot_self": sself,
            "slot_partner": spart,
        })
    return in_maps


def run(q, k, v, w_q, w_k, w_v, w_p, trace=False, tmpdir=None):
    if "nc" not in _cache:
        _cache["nc"] = build()
    nc = _cache["nc"]
    in_maps = _in_maps(q, k, v, w_q, w_k, w_v, w_p)
    res = run_bass_kernel_spmd(nc, in_maps, core_ids=list(range(NCORES)),
                               trace=trace, tmpdir=tmpdir)
    out = np.empty((B, S, H), np.float32)
    for c in range(NCORES):
        b, h = c // 2, c % 2
        out[b, h * SC_CORE:(h + 1) * SC_CORE, :] = res.results[c]["out"]
    return out, res


def kernel(**inputs):
    out, _ = run(**{k: np.asarray(v) for k, v in inputs.items()})
    return out
